# revision 21
# baseline (speedup 1.0000x reference)
"""EME loss kernel for Trainium2, 8 NeuronCores, pure data-parallel.

Math (matches the jax reference):
  y_pred [32, 3, 1024, 1024] f32; 8x8 non-overlapping window max/min pooling;
  vals = 20*ln(max/(min+1e-4)); per_batch = sum(vals)/(1024*1024)*64;
  out = mean(per_batch) -> f32 scalar.

Sharding: batch across 8 cores (4 batches = 12 images of 1024x1024 per core).
Device computes per-partition partial sums of (ln(max) - ln(min+eps)); host
combines: out = total * 20 * 64 / 2^20 / 32.

Staging: input cast to bf16 on the host (round-to-nearest via ml_dtypes),
halving HBM traffic vs fp32 -- the kernel computed in bf16 anyway and the
rel-err budget is 2e-2 (measured ~8e-6 for bf16). Loads use plain HWDGE
(sync-engine) DMA; 24 MiB/core at ~358 GB/s/NC = ~70 us.

Layout: a 1024x1024 image viewed as [128, 8192] puts one window-row
(8 image rows, 16KB bf16 contiguous) on each partition; free idx =
r*1024 + w*8 + j (r=row-in-window, w=window, j=col-in-window).

Compute: pairwise max/min trees on DVE over _G-image groups (bf16
tensor_tensor runs 2x = 2 elem/cycle/partition; measured 0.52-0.56
ns/elem). DVE busy ~109 us is the bottleneck (DMA ~70 us). Grouping
images amortizes the ~160ns per-instruction overhead. Other engines
cannot help: Pool TT supports only add/sub in codegen, runs ~1.4-1.8
ns/elem, and its SBUF traffic throttles concurrent DVE 2x ops up to
2.4x (measured 202us with a Pool offload vs 130us without); Scalar/ACT
is unary; PE has no compare; DMA CCE accumulate supports add only.
"""
import numpy as np
import concourse.bass as bass
import concourse.mybir as mybir
import concourse.tile as tile
from concourse.bass_utils import run_bass_kernel_spmd

_N_CORES = 8
_B, _C, _H, _W = 32, 3, 1024, 1024
_IMGS_PER_CORE = (_B // _N_CORES) * _C  # 12
_G = 3  # images per compute group
_GROUPS = _IMGS_PER_CORE // _G
_WIN = 8
_EPS = 1e-4

_NC_CACHE = {}
LAST_RESULTS = None  # BassKernelResults of the most recent run (for test.py)


def _split_excess_waits(nc, max_waits=1):
    """This walrus build rejects >2 sync-waits on one CTRL instruction (the
    Tile exit drain collects one wait per active logical proc). Move excess
    waits onto preceding NoOps on the same engine."""
    for func in nc.m.functions:
        for bb in func.blocks:
            insts = bb.instructions
            out_insts = []
            changed = False
            for ins in insts:
                si = getattr(ins, "sync_info", None)
                if si is not None and si.on_wait and len(si.on_wait) > max_waits:
                    waits = list(si.on_wait)
                    head, tail = waits[:-max_waits], waits[-max_waits:]
                    for j in range(0, len(head), max_waits):
                        nop = mybir.InstNoOp(name=f"{ins.name}-wsplit{j}", ins=[], outs=[])
                        nop.engine = ins.engine
                        nop.sync_info = mybir.SyncInfo(
                            on_wait=head[j:j + max_waits], on_update=[])
                        out_insts.append(nop)
                    ins.sync_info = mybir.SyncInfo(on_wait=tail, on_update=si.on_update)
                    changed = True
                out_insts.append(ins)
            if changed:
                bb.instructions = out_insts


def _light_drain_and_barrier(self, tick_clock, wait_clock):
    """TileContext exit ceremony minus the trailing all-engine barrier
    (drain already waits on the global clock; NEFF completion waits on all
    engine programs regardless). Saves a few us of kernel-exit time."""
    from concourse.vector_clock import ScopedClock
    drain_inst = self.nc.sync.drain()
    wait_clock.add_sem_waits(drain_inst.ins,
                             ScopedClock({None: tick_clock.global_clock}))
    self.nc.all_engine_barrier()
    popped = self.nc._tile_sem_poison_stack.pop()
    assert popped is self._sem_poison
    self.nc._state.prepend_free_semaphores(
        [s.num if hasattr(s, "num") else s for s in self.sems.allocated().values()])


def _build():
    F32 = mybir.dt.float32
    BF16 = mybir.dt.bfloat16
    nc = bass.Bass()
    y = nc.declare_dram_parameter("y", [_IMGS_PER_CORE, _H, _W], BF16,
                                  isOutput=False)
    out = nc.declare_dram_parameter("out", [1, 1], F32, isOutput=True)

    MAX, MIN = mybir.AluOpType.max, mybir.AluOpType.min
    G = _G

    tile.TileContext._drain_and_barrier = _light_drain_and_barrier
    with tile.TileContext(nc) as tc:
        with tc.tile_pool(name="img", bufs=2) as img_pool, \
             tc.tile_pool(name="l1", bufs=1) as l1_pool, \
             tc.tile_pool(name="tv", bufs=1) as tv_pool, \
             tc.tile_pool(name="tx", bufs=2) as tx_pool, \
             tc.tile_pool(name="stat", bufs=2) as stat_pool, \
             tc.tile_pool(name="accp", bufs=1) as acc_pool, \
             tc.tile_pool(name="psum", bufs=1, space="PSUM") as psum_pool:
            partsP = acc_pool.tile([128, _GROUPS], F32, tag="partsP")
            partsN = acc_pool.tile([128, _GROUPS], F32, tag="partsN")
            # eps bias for ln(min + eps), written on DVE (tile-tracked, no
            # global barrier needed; gpsimd memset would cost a Q7 launch
            # plus an all-engine barrier at kernel start)
            epsb = acc_pool.tile([128, 1], F32, tag="epsb")
            nc.vector.memset(epsb[:], _EPS)

            def htree(which, cur, op):
                """Horizontal j=8->1 on [128, G*1024] (= [i, w, j8]) ->
                [128, G*128]."""
                src, width = cur, G * 1024
                for jj in (4, 2, 1):
                    v = src[:].rearrange("p (i w k) -> p i w k", i=G, k=2 * jj)
                    pool = tx_pool if jj == 1 else tv_pool
                    nxt = pool.tile([128, width // 2], BF16, tag=f"{which}h{jj}")
                    nv = nxt[:].rearrange("p (i w k) -> p i w k", i=G, k=jj)
                    nc.vector.tensor_tensor(out=nv, in0=v[:, :, :, 0:jj],
                                            in1=v[:, :, :, jj:2 * jj], op=op)
                    src, width = nxt, width // 2
                return src  # [128, G*128] = (i, w)

            for k in range(_GROUPS):
                T = img_pool.tile([128, G * 8192], BF16, tag="img")
                first = k == 0
                if first:
                    # kernel warmup: image 0 lands as 4 contiguous 512-KB
                    # chunks; max/min pairing is arbitrary, so each chunk
                    # (rows 2q, 2q+1) unlocks one half-sized L1 op pairing
                    # those adjacent rows -- downstream levels are unchanged.
                    # (Strided 512-KB chunks were tried and do NOT start
                    # earlier -- they process slower through the queue ramp.)
                    src0 = y[G * k].rearrange("(p r) c -> p (r c)", p=128)
                    for q in range(4):
                        nc.sync.dma_start(out=T[:, q * 2048:(q + 1) * 2048],
                                          in_=src0[:, q * 2048:(q + 1) * 2048])
                    for i in range(1, G):
                        src = y[G * k + i].rearrange("(p r) c -> p (r c)", p=128)
                        nc.sync.dma_start(out=T[:, i * 8192:i * 8192 + 4096],
                                          in_=src[:, 0:4096])
                        nc.sync.dma_start(out=T[:, i * 8192 + 4096:(i + 1) * 8192],
                                          in_=src[:, 4096:8192])
                else:
                    for i in range(G):
                        src = y[G * k + i].rearrange("(p r) c -> p (r c)", p=128)
                        nc.sync.dma_start(out=T[:, i * 8192:i * 8192 + 4096],
                                          in_=src[:, 0:4096])
                    for i in range(G):
                        src = y[G * k + i].rearrange("(p r) c -> p (r c)", p=128)
                        nc.sync.dma_start(out=T[:, i * 8192 + 4096:(i + 1) * 8192],
                                          in_=src[:, 4096:8192])
                # W: [p, i(img), g(top/bot), h(row-pair), e]; e = parity*1024+w*8+j
                W = T[:].rearrange("p (i g h e) -> p i g h e", i=G, g=2, e=2048)
                mxa = l1_pool.tile([128, G * 4096], BF16, tag="mxa")  # [i, g, e]
                mna = l1_pool.tile([128, G * 4096], BF16, tag="mna")
                mxav = mxa[:].rearrange("p (i g e) -> p i g e", i=G, g=2)
                mnav = mna[:].rearrange("p (i g e) -> p i g e", i=G, g=2)
                if first:
                    # image 0: adjacent-row pair ops chase the 512-KB chunks;
                    # writes [a_{2q} v a_{2q+1}] into the same mxa/mna slots
                    # (c/cur reduce them identically regardless of pairing)
                    for q in range(4):
                        o0, o1 = q * 1024, (q + 1) * 1024
                        nc.vector.tensor_tensor(
                            out=mxa[:, o0:o1], in0=T[:, 2 * o0:2 * o0 + 1024],
                            in1=T[:, 2 * o0 + 1024:2 * o1], op=MAX)
                        nc.vector.tensor_tensor(
                            out=mna[:, o0:o1], in0=T[:, 2 * o0:2 * o0 + 1024],
                            in1=T[:, 2 * o0 + 1024:2 * o1], op=MIN)
                    # remaining images: per-image, per-half L1 ops
                    for i in range(1, G):
                        for g in range(2):
                            nc.vector.tensor_tensor(out=mxav[:, i:i + 1, g, :],
                                                    in0=W[:, i:i + 1, g, 0, :],
                                                    in1=W[:, i:i + 1, g, 1, :],
                                                    op=MAX)
                            nc.vector.tensor_tensor(out=mnav[:, i:i + 1, g, :],
                                                    in0=W[:, i:i + 1, g, 0, :],
                                                    in1=W[:, i:i + 1, g, 1, :],
                                                    op=MIN)
                else:
                    for g in range(2):  # top rows (0-3) first; bottom after
                        nc.vector.tensor_tensor(out=mxav[:, :, g, :],
                                                in0=W[:, :, g, 0, :],
                                                in1=W[:, :, g, 1, :], op=MAX)
                        nc.vector.tensor_tensor(out=mnav[:, :, g, :],
                                                in0=W[:, :, g, 0, :],
                                                in1=W[:, :, g, 1, :], op=MIN)
                trees = []
                for which, a, op in (("mx", mxa, MAX), ("mn", mna, MIN)):
                    av = a[:].rearrange("p (i g e) -> p i g e", i=G, g=2)
                    c = tv_pool.tile([128, G * 2048], BF16, tag=f"{which}c")
                    cv = c[:].rearrange("p (i e) -> p i e", i=G)
                    nc.vector.tensor_tensor(out=cv, in0=av[:, :, 0, :],
                                            in1=av[:, :, 1, :], op=op)
                    cc = c[:].rearrange("p (i h e) -> p i h e", i=G, h=2)
                    cur = tv_pool.tile([128, G * 1024], BF16, tag=f"{which}v")
                    nc.vector.tensor_tensor(
                        out=cur[:].rearrange("p (i e) -> p i e", i=G),
                        in0=cc[:, :, 0, :], in1=cc[:, :, 1, :], op=op)
                    trees.append(htree(which, cur, op))
                mx, mn = trees
                lmx = stat_pool.tile([128, G * 128], F32, tag="lmx")
                lmn = stat_pool.tile([128, G * 128], F32, tag="lmn")
                nc.scalar.activation(lmx[:], mx[:], mybir.ActivationFunctionType.Ln,
                                     accum_out=partsP[:, k:k + 1])
                nc.scalar.activation(lmn[:], mn[:], mybir.ActivationFunctionType.Ln,
                                     bias=epsb[:], accum_out=partsN[:, k:k + 1])
            # warm the HWDGE ring used by the final out-DMA so it doesn't pay
            # first-use latency on the critical-path tail (~2.5 us). Issued
            # AFTER all load dma_starts: at program start it delays the load
            # queue spin-up by ~4 us (measured).
            warm = acc_pool.tile([1, 1], BF16, tag="warm")
            nc.sync.dma_start(out=warm[:], in_=y[0, 0:1, 0:1])
            aP = acc_pool.tile([128, 1], F32, tag="aP")
            nc.vector.tensor_reduce(out=aP[:], in_=partsP[:],
                                    axis=mybir.AxisListType.X,
                                    op=mybir.AluOpType.add)
            aN = acc_pool.tile([128, 1], F32, tag="aN")
            nc.vector.tensor_reduce(out=aN[:], in_=partsN[:],
                                    axis=mybir.AxisListType.X,
                                    op=mybir.AluOpType.add)
            acc = acc_pool.tile([128, 1], F32, tag="acc")
            nc.vector.tensor_tensor(out=acc[:], in0=aP[:], in1=aN[:],
                                    op=mybir.AluOpType.subtract)
            # collapse partitions with a 1x128 @ 128x1 matmul so the out-DMA
            # is a single descriptor (a [128,1] DMA = 128 tiny descriptors)
            ones = nc.const_aps.tensor(1.0, (128, 1))
            pt = psum_pool.tile([1, 1], F32, tag="pt")
            nc.tensor.matmul(pt[:], acc[:], ones)
            total = acc_pool.tile([1, 1], F32, tag="total")
            nc.vector.tensor_copy(out=total[:], in_=pt[:])
            nc.sync.dma_start(out=out[:], in_=total[:])

    # Bass init registers const tensors with gpsimd.memset; the Q7 cold
    # start (~3 us) then stalls the first all-engine barrier. Run those
    # memsets on DVE instead (legal, and DVE is up immediately).
    for bb in nc.m.functions[0].blocks:
        for ins in bb.instructions:
            if ins.__class__.__name__ == "InstDrain":
                break
            if (ins.__class__.__name__ == "InstMemset"
                    and ins.engine == mybir.EngineType.Pool):
                ins.engine = mybir.EngineType.DVE
        break

    _split_excess_waits(nc)
    return nc


def _get_nc():
    if "nc" not in _NC_CACHE:
        _NC_CACHE["nc"] = _build()
    return _NC_CACHE["nc"]


def kernel(y_pred, winSize=8, _trace=False, **_ignored):
    global LAST_RESULTS
    assert int(winSize) == _WIN
    bf16 = mybir.dt.np(mybir.dt.bfloat16)
    y = np.ascontiguousarray(np.asarray(y_pred, dtype=np.float32)).astype(bf16)
    assert y.shape == (_B, _C, _H, _W)
    per_core_b = _B // _N_CORES
    in_maps = [
        {"y": y[c * per_core_b:(c + 1) * per_core_b].reshape(_IMGS_PER_CORE, _H, _W)}
        for c in range(_N_CORES)
    ]
    nc = _get_nc()
    res = run_bass_kernel_spmd(nc, in_maps, list(range(_N_CORES)), trace=_trace)
    LAST_RESULTS = res
    total = np.sum([float(r["out"][0, 0]) for r in res.results])
    val = total * 20.0 * (_WIN * _WIN) / (_H * _W) / _B
    return np.float32(val)
